# revision 1
# baseline (speedup 1.0000x reference)
"""3D Haar DWT (depth-1) Trainium2 kernel.

Full inputs: x [4, 4, 64, 256, 256] f32 + six banded Haar matrices
(hardcoded math: every output element is +-2^-1.5 times a +-sum of a
2x2x2 block). Returns the 8 subbands (LLL, LLH, LHL, LHH, HLL, HLH,
HHL, HHH), each [4, 4, 32, 128, 128] f32.

Sharding: data-parallel over N*C = 16 sample-channels, 2 per core on
8 cores. Per-core compute is a 3-stage butterfly over pair-packed
tiles (SBUF partition p holds input rows 2p and 2p+1 contiguously, so
every DMA descriptor is a 2 KiB linear run):
  H stage: row pairs    -> TensorE matmuls against +-2^-1.5 * I for
                           3 of every 4 d-pairs (fp32, exact);
                           DVE adds + ScalarE pre-scale for the 4th.
  W stage: column pairs -> DVE stride-2 tensor_add/sub (FD=1024)
  D stage: slice pairs  -> DVE tensor_add/sub (FD=1024, 4-D APs
                           covering two subbands per instruction)
ScalarE evacuates PSUM. Everything stays fp32-exact.
"""
import sys

sys.path.insert(0, "/opt/trn_rl_repo")

import numpy as np

N, C, D, H, W = 4, 4, 64, 256, 256
NCORES = 8
G_PER_CORE = (N * C) // NCORES        # 2
KP = D // 2                           # 32 d-pairs per g
S3 = float(2.0 ** -1.5)

# schedule tunables
KB = 8                                # k-slices per output staging block
IN_BUFS = 8
EV_BUFS = 4
WT_BUFS = 2
OS_BUFS = 2
PSUM_BUFS = 3

_CACHE = {}


def _build_filter_lhst():
    """Stationary operands: +S3*I and -S3*I, as [2, 128, 128] fp32."""
    eye = np.eye(128, dtype=np.float32)
    return np.stack([np.float32(S3) * eye, np.float32(-S3) * eye])


def _build_nc():
    import concourse.bass as bass
    import concourse.tile as tile
    from concourse import bacc, mybir

    f32 = mybir.dt.float32
    nc = bacc.Bacc(None)
    x_d = nc.declare_dram_parameter("x", [G_PER_CORE, D, H, W], f32,
                                    isOutput=False)
    ft_d = nc.declare_dram_parameter("ft", [2, 128, 128], f32,
                                     isOutput=False)
    # h'-major layout: per (s, g, partition=h') a k-block of 8 is one
    # contiguous 4 KiB run in DRAM (host transposes k and h' back)
    o_d = nc.declare_dram_parameter("out", [8, G_PER_CORE, 128, KP, 128],
                                    f32, isOutput=True)

    with tile.TileContext(nc) as tc:
        with (
            tc.tile_pool(name="cst", bufs=1) as cst,
            tc.tile_pool(name="inp", bufs=IN_BUFS) as inp,
            tc.tile_pool(name="ev", bufs=EV_BUFS) as evp,
            tc.tile_pool(name="wt", bufs=WT_BUFS) as wtp,
            tc.tile_pool(name="os", bufs=OS_BUFS) as osp,
            tc.tile_pool(name="ps", bufs=PSUM_BUFS, space="PSUM") as psp,
        ):
            ft = cst.tile([128, 256], f32, tag="ft")
            nc.sync.dma_start(
                ft.rearrange("p (i c) -> p i c", i=2),
                ft_d.rearrange("i p c -> p i c"))
            pos_i = ft[:, 0:128]    # +S3 * I
            neg_i = ft[:, 128:256]  # -S3 * I

            def load_pair(g, k):
                """One d-pair as a pair-packed tile [128, 1024]:
                cols = {s0: row2p row2p+1 | s1: row2p row2p+1}."""
                t = inp.tile([128, 1024], f32, tag="xin")
                nc.sync.dma_start(
                    t.rearrange("p (s r) -> p s r", s=2),
                    x_d[g, 2 * k:2 * k + 2].rearrange(
                        "s (p r) w -> p s (r w)", r=2))
                return t

            for g in range(G_PER_CORE):
                for kb in range(KP // KB):
                    os_t = osp.tile([128, 8 * KB * 128], f32, tag="os")
                    for half in range(KB // 4):
                        wt_t = wtp.tile([128, 4 * 1024], f32, tag="wt")
                        # EV tiles: j0+j1 (both PE), j2 (PE) + j3 (DVE)
                        ev01 = evp.tile([128, 2048], f32, tag="ev")
                        ev23 = evp.tile([128, 2048], f32, tag="ev")
                        for j in range(4):
                            k = kb * KB + half * 4 + j
                            t = load_pair(g, k)
                            t4 = t.rearrange("p (s r w) -> p s r w",
                                             s=2, r=2)
                            if j < 3:
                                # --- H stage on TensorE: +-S3*I matmuls
                                pt = psp.tile([128, 1024], f32, tag="ps")
                                lo = pt[:, 0:512].rearrange(
                                    "p (s w) -> p s w", s=2)
                                hi = pt[:, 512:1024].rearrange(
                                    "p (s w) -> p s w", s=2)
                                nc.tensor.matmul(lo, pos_i, t4[:, :, 0, :],
                                                 start=True, stop=False)
                                nc.tensor.matmul(lo, pos_i, t4[:, :, 1, :],
                                                 start=False, stop=True)
                                nc.tensor.matmul(hi, pos_i, t4[:, :, 0, :],
                                                 start=True, stop=False)
                                nc.tensor.matmul(hi, neg_i, t4[:, :, 1, :],
                                                 start=False, stop=True)
                                # ScalarE evacuation (scaled via weights)
                                dst = (ev01[:, j * 1024:(j + 1) * 1024]
                                       if j < 2 else ev23[:, 0:1024])
                                nc.scalar.activation(
                                    dst, pt[:],
                                    mybir.ActivationFunctionType.Copy)
                            else:
                                # --- H stage on DVE (ScalarE pre-scale)
                                nc.scalar.activation(
                                    t[:], t[:],
                                    mybir.ActivationFunctionType.Copy,
                                    bias=0.0, scale=S3)
                                pl = ev23[:, 1024:2048]
                                pl3 = pl.rearrange("p (b s w) -> p b s w",
                                                   b=2, s=2)
                                nc.vector.tensor_add(
                                    pl3[:, 0], t4[:, :, 0, :],
                                    t4[:, :, 1, :])
                                nc.vector.tensor_sub(
                                    pl3[:, 1], t4[:, :, 0, :],
                                    t4[:, :, 1, :])
                        # --- W stage on DVE, FD=1024 ---
                        # ev layout per 1024: {A_lo(s0,s1) | A_hi(s0,s1)}
                        wt4 = wt_t.rearrange("p (j b) -> p j b", j=4)
                        for ev, j0 in ((ev01, 0), (ev23, 2)):
                            nc.vector.tensor_add(
                                wt4[:, j0:j0 + 2, 0:512],
                                ev[:, 0::2].rearrange(
                                    "p (j b) -> p j b", j=2),
                                ev[:, 1::2].rearrange(
                                    "p (j b) -> p j b", j=2))
                            nc.vector.tensor_sub(
                                wt4[:, j0:j0 + 2, 512:1024],
                                ev[:, 0::2].rearrange(
                                    "p (j b) -> p j b", j=2),
                                ev[:, 1::2].rearrange(
                                    "p (j b) -> p j b", j=2))
                        # --- D stage, FD=1024, two subbands per op ---
                        # wt_t per-pair block (j): {LL0 LL1 HL0 HL1 |
                        #                           LH0 LH1 HH0 HH1}
                        wtd = wt_t.rearrange("p (j c w) -> p c j w",
                                             j=4, c=8)
                        osd = os_t.rearrange("p (s q w) -> p s q w",
                                             s=8, q=KB)
                        qs = slice(half * 4, half * 4 + 4)
                        for c0, s_sum, s_diff in ((0, 0, 4), (4, 1, 5)):
                            # c blocks {c0, c0+2} = {LL,HL} / {LH,HH}
                            in0 = wtd[:, c0:c0 + 3:2]
                            in1 = wtd[:, c0 + 1:c0 + 4:2]
                            nc.vector.tensor_add(
                                osd[:, s_sum:s_sum + 3:2, qs], in0, in1)
                            nc.vector.tensor_sub(
                                osd[:, s_diff:s_diff + 3:2, qs], in0, in1)
                    # --- store this k-block: 8 subbands x [128,KB,128] ---
                    for s in range(8):
                        src_ap = os_t[:, s * KB * 128:(s + 1) * KB * 128]
                        nc.sync.dma_start(
                            o_d[s, g, :, kb * KB:(kb + 1) * KB, :],
                            src_ap.rearrange("p (q w) -> p q w", q=KB))
    nc.finalize()
    return nc


def _get_nc():
    if "nc" not in _CACHE:
        _CACHE["nc"] = _build_nc()
    return _CACHE["nc"]


def kernel(x, low_0, low_1, low_2, high_0, high_1, high_2):
    from concourse.bass_utils import run_bass_kernel_spmd

    x = np.ascontiguousarray(np.asarray(x, dtype=np.float32))
    ft = _build_filter_lhst()
    xs = x.reshape(N * C, D, H, W)
    in_maps = [
        {"x": np.ascontiguousarray(
            xs[c * G_PER_CORE:(c + 1) * G_PER_CORE]), "ft": ft}
        for c in range(NCORES)
    ]
    nc = _get_nc()
    res = run_bass_kernel_spmd(nc, in_maps, list(range(NCORES)))
    full = np.empty((8, N * C, KP, 128, 128), dtype=np.float32)
    for c in range(NCORES):
        full[:, c * G_PER_CORE:(c + 1) * G_PER_CORE] = \
            res.results[c]["out"].transpose(0, 1, 3, 2, 4)
    full = full.reshape(8, N, C, KP, 128, 128)
    return tuple(full[s] for s in range(8))



# revision 8
# speedup vs baseline: 1.9985x; 1.9985x over previous
"""3D Haar DWT (depth-1) Trainium2 kernel, fp16 dataflow.

Full inputs: x [4, 4, 64, 256, 256] f32 + six banded Haar matrices.
Every output element is +-2^-1.5 times a +-sum of a 2x2x2 block; the
kernel computes the 3-stage butterfly in fp16 (rel err ~2e-3 vs the
2e-2 gate) to halve DMA traffic, which is the binding resource
(16 DMA engines x 22.5 GB/s per core).

Sharding: data-parallel over N*C = 16 sample-channels, 2 per core on
8 cores. The host pre-transposes each core's input into the exact
SBUF tile layout [g, tile, h', kpar, s, r, w] so every DMA descriptor
is a 4 KiB contiguous run, and un-transposes the packed output
[g, h', kb, sb, q, w'] (16 KiB runs) afterwards.

Per-core pipeline, per 2-pair tile:
  H stage  (DVE):     row pairs r0+-r1, unit-stride fp16 16-bit mode
  W stage  (TensorE): column pairs via accumulating matmuls against
                      +-2^-1.5 * I fp16 stationaries (scale folded in)
  evac     (ScalarE): PSUM -> SBUF fp16 downcast
  D stage  (DVE):     slice pairs s0+-s1 batched over 8 d-pairs
"""
import sys

sys.path.insert(0, "/opt/trn_rl_repo")

import numpy as np

N, C, D, H, W = 4, 4, 64, 256, 256
NCORES = 8
G_PER_CORE = (N * C) // NCORES        # 2
KP = D // 2                           # 32 d-pairs per g
S3 = float(2.0 ** -1.5)

PPT = 2                               # d-pairs per input tile
NT = KP // PPT                        # input tiles per g (16)
KB = 8                                # d-pairs per output staging block
NKB = KP // KB                        # staging blocks per g (4)
TPB = KB // PPT                       # input tiles per staging block (4)

IN_BUFS = 4
A_BUFS = 3
PSUM_BUFS = 2
EV_BUFS = 2
OS_BUFS = 2

_CACHE = {}


def _build_filter_lhst():
    """Stationary operands +-2^-1.5 * I as [2, 128, 128] fp16."""
    eye = np.eye(128, dtype=np.float16)
    return np.stack([np.float16(S3) * eye, -np.float16(S3) * eye])


def _shard_input(x):
    """x [N, C, D, H, W] f32 -> list of per-core [G, NT, 128, PPT*1024]
    fp16 arrays laid out [g, tile, h', kpar, s, r, w]."""
    xs = np.asarray(x).reshape(N * C, D, H, W)
    out = []
    for c in range(NCORES):
        xc = xs[c * G_PER_CORE:(c + 1) * G_PER_CORE]
        # d = tile*2*PPT + kpar*2 + s ; h = h'*2 + r
        v = xc.reshape(G_PER_CORE, NT, PPT, 2, 128, 2, W)
        v = v.transpose(0, 1, 4, 2, 3, 5, 6).astype(np.float16)
        out.append(np.ascontiguousarray(
            v.reshape(G_PER_CORE, NT, 128, PPT * 2 * 2 * W)))
    return out


def _build_nc():
    import concourse.bass as bass
    import concourse.tile as tile
    from concourse import bacc, mybir

    f16 = mybir.dt.float16
    f32 = mybir.dt.float32
    nc = bacc.Bacc(None)
    x_d = nc.declare_dram_parameter("x", [G_PER_CORE, NT, 128, 2048], f16,
                                    isOutput=False)
    ft_d = nc.declare_dram_parameter("ft", [2, 128, 128], f16,
                                     isOutput=False)
    # packed output: [g, h', kb, (sb q w')] -> 16 KiB run per partition
    o_d = nc.declare_dram_parameter("out", [G_PER_CORE, 128, NKB, 8192],
                                    f16, isOutput=True)

    with tile.TileContext(nc) as tc:
        with (
            tc.tile_pool(name="cst", bufs=1) as cst,
            tc.tile_pool(name="inp", bufs=IN_BUFS) as inp,
            tc.tile_pool(name="hout", bufs=A_BUFS) as hop,
            tc.tile_pool(name="ev", bufs=EV_BUFS) as evp,
            tc.tile_pool(name="os", bufs=OS_BUFS) as osp,
            tc.tile_pool(name="ps", bufs=PSUM_BUFS, space="PSUM") as psp,
        ):
            ft = cst.tile([128, 256], f16, tag="ft")
            nc.sync.dma_start(
                ft.rearrange("p (i c) -> p i c", i=2),
                ft_d.rearrange("i p c -> p i c"))
            pos_i = ft[:, 0:128]    # +S3 * I
            neg_i = ft[:, 128:256]  # -S3 * I

            for g in range(G_PER_CORE):
                for kb in range(NKB):
                    # ev: evacuated W-stage results for one KB block,
                    # layout (s, it, k, e, c, f) so the D stage is two
                    # fully contiguous 4096-elem DVE ops
                    ev = evp.tile([128, KB * 1024], f16, tag="ev")
                    ev7 = ev.rearrange(
                        "p (s it k e c f) -> p it k e s c f",
                        s=2, it=TPB, k=PPT, e=2, c=2)
                    for it in range(TPB):
                        t = inp.tile([128, 2048], f16, tag="xin")
                        nc.sync.dma_start(t[:], x_d[g, kb * TPB + it])
                        # --- H stage on DVE ---
                        a = hop.tile([128, 2048], f16, tag="a")
                        t5 = t.rearrange("p (k s r w) -> p k s r w",
                                         k=PPT, s=2, r=2)
                        a5 = a.rearrange("p (k c s w) -> p k c s w",
                                         k=PPT, c=2, s=2)
                        nc.vector.tensor_add(
                            a5[:, :, 0], t5[:, :, :, 0], t5[:, :, :, 1])
                        nc.vector.tensor_sub(
                            a5[:, :, 1], t5[:, :, :, 0], t5[:, :, :, 1])
                        # --- W stage on TensorE ---
                        # P layout (k, e, s, c, f): each matmul target
                        # (kpar, e) is one contiguous 512-elem PSUM bank
                        pt = psp.tile([128, 2048], f32, tag="ps")
                        a7 = a.rearrange(
                            "p (k c s wq f) -> p k s c wq f",
                            k=PPT, c=2, s=2, wq=128)
                        p6 = pt.rearrange(
                            "p (k e s c f) -> p k e s c f",
                            k=PPT, e=2, s=2, c=2)
                        for kpar in range(PPT):
                            rhs_e = a7[:, kpar, :, :, :, 0]
                            rhs_o = a7[:, kpar, :, :, :, 1]
                            lo = pt[:, kpar * 1024:kpar * 1024 + 512]
                            hi = pt[:, kpar * 1024 + 512:(kpar + 1) * 1024]
                            nc.tensor.matmul(lo, pos_i, rhs_e,
                                             start=True, stop=False)
                            nc.tensor.matmul(lo, pos_i, rhs_o,
                                             start=False, stop=True)
                            nc.tensor.matmul(hi, pos_i, rhs_e,
                                             start=True, stop=False)
                            nc.tensor.matmul(hi, neg_i, rhs_o,
                                             start=False, stop=True)
                        # --- evac PSUM -> SBUF fp16 (ScalarE) ---
                        # one op per kpar: in contiguous, out 3 loops
                        for kpar in range(PPT):
                            nc.scalar.activation(
                                ev7[:, it, kpar], p6[:, kpar],
                                mybir.ActivationFunctionType.Copy)
                    # --- D stage on DVE: two contiguous 4096-elem ops
                    sd = osp.tile([128, 8192], f16, tag="sd")
                    nc.vector.tensor_add(
                        sd[:, 0:4096], ev[:, 0:4096], ev[:, 4096:8192])
                    nc.vector.tensor_sub(
                        sd[:, 4096:8192], ev[:, 0:4096], ev[:, 4096:8192])
                    nc.sync.dma_start(o_d[g, :, kb], sd[:])
    nc.finalize()
    return nc


def _get_nc():
    if "nc" not in _CACHE:
        _CACHE["nc"] = _build_nc()
    return _CACHE["nc"]


def _unshard_output(results):
    """results: per-core [G, 128, NKB, 8192] fp16 arrays where the
    8192 block is (d, it, k, e, c, f) -> 8 subbands (sb = d*4+c*2+e)
    of [N, C, D/2, H/2, W/2] f32."""
    full = np.stack([np.asarray(r).reshape(
        G_PER_CORE, 128, NKB, 2, TPB, PPT, 2, 2, 128) for r in results])
    # [co, g, h', kb, d, it, k, e, c, f] -> [d, c, e, co, g, kb, it, k, h', f]
    full = full.transpose(4, 8, 7, 0, 1, 3, 5, 6, 2, 9)
    full = full.reshape(8, N, C, KP, 128, 128).astype(np.float32)
    return full


def kernel(x, low_0, low_1, low_2, high_0, high_1, high_2):
    from concourse.bass_utils import run_bass_kernel_spmd

    ft = _build_filter_lhst()
    shards = _shard_input(x)
    in_maps = [{"x": shards[c], "ft": ft} for c in range(NCORES)]
    nc = _get_nc()
    res = run_bass_kernel_spmd(nc, in_maps, list(range(NCORES)))
    full = _unshard_output([res.results[c]["out"] for c in range(NCORES)])
    return tuple(full[s] for s in range(8))


# revision 11
# speedup vs baseline: 2.1595x; 1.0806x over previous
"""3D Haar DWT (depth-1) Trainium2 kernel, fp16 dataflow.

Full inputs: x [4, 4, 64, 256, 256] f32 + six banded Haar matrices.
Every output element is +-2^-1.5 times a +-sum of a 2x2x2 block; the
kernel computes the 3-stage butterfly in fp16 (rel err ~2e-3 vs the
2e-2 gate) to halve DMA traffic, which is the binding resource
(16 DMA engines x 22.5 GB/s per core).

Sharding: data-parallel over N*C = 16 sample-channels, 2 per core on
8 cores. The host pre-transposes each core's input into the exact
SBUF tile layout [g, tile, h', kpar, s, r, w] so every DMA descriptor
is a 4 KiB contiguous run, and un-transposes the packed output
[g, h', kb, sb, q, w'] (16 KiB runs) afterwards.

Per-core pipeline, per 2-pair tile:
  H stage  (DVE):     row pairs r0+-r1, unit-stride fp16 16-bit mode
  W stage  (TensorE): column pairs via accumulating matmuls against
                      +-2^-1.5 * I fp16 stationaries (scale folded in)
  evac     (ScalarE): PSUM -> SBUF fp16 downcast
  D stage  (DVE):     slice pairs s0+-s1 batched over 8 d-pairs
"""
import sys

sys.path.insert(0, "/opt/trn_rl_repo")

import numpy as np

N, C, D, H, W = 4, 4, 64, 256, 256
NCORES = 8
G_PER_CORE = (N * C) // NCORES        # 2
KP = D // 2                           # 32 d-pairs per g
S3 = float(2.0 ** -1.5)

PPT = 2                               # d-pairs per input tile
NT = KP // PPT                        # input tiles per g (16)
KB = 8                                # d-pairs per output staging block
NKB = KP // KB                        # staging blocks per g (4)
TPB = KB // PPT                       # input tiles per staging block (4)

IN_BUFS = 6
A_BUFS = 4
PSUM_BUFS = 4
EV_BUFS = 2
OS_BUFS = 4

_CACHE = {}


def _build_filter_lhst():
    """Stationary operands +-2^-1.5 * I as [2, 128, 128] fp16."""
    eye = np.eye(128, dtype=np.float16)
    return np.stack([np.float16(S3) * eye, -np.float16(S3) * eye])


def _shard_input(x):
    """x [N, C, D, H, W] f32 -> list of per-core [G, NT, 128, PPT*1024]
    fp16 arrays laid out [g, tile, h', kpar, s, r, w]."""
    xs = np.asarray(x).reshape(N * C, D, H, W)
    out = []
    for c in range(NCORES):
        xc = xs[c * G_PER_CORE:(c + 1) * G_PER_CORE]
        # d = tile*2*PPT + kpar*2 + s ; h = h'*2 + r
        v = xc.reshape(G_PER_CORE, NT, PPT, 2, 128, 2, W)
        v = v.transpose(0, 1, 4, 2, 3, 5, 6).astype(np.float16)
        out.append(np.ascontiguousarray(
            v.reshape(G_PER_CORE, NT, 128, PPT * 2 * 2 * W)))
    return out


def _build_nc():
    import concourse.bass as bass
    import concourse.tile as tile
    from concourse import bacc, mybir

    f16 = mybir.dt.float16
    f32 = mybir.dt.float32
    nc = bacc.Bacc(None)
    x_d = nc.declare_dram_parameter("x", [G_PER_CORE, NT, 128, 2048], f16,
                                    isOutput=False)
    ft_d = nc.declare_dram_parameter("ft", [2, 128, 128], f16,
                                     isOutput=False)
    # packed output: [g, h', kb, (sb q w')] -> 16 KiB run per partition
    o_d = nc.declare_dram_parameter("out", [G_PER_CORE, 128, NKB, 8192],
                                    f16, isOutput=True)

    with tile.TileContext(nc) as tc:
        with (
            tc.tile_pool(name="cst", bufs=1) as cst,
            tc.tile_pool(name="inp", bufs=IN_BUFS) as inp,
            tc.tile_pool(name="hout", bufs=A_BUFS) as hop,
            tc.tile_pool(name="ev", bufs=EV_BUFS) as evp,
            tc.tile_pool(name="os", bufs=OS_BUFS) as osp,
            tc.tile_pool(name="ps", bufs=PSUM_BUFS, space="PSUM") as psp,
        ):
            ft = cst.tile([128, 256], f16, tag="ft")
            nc.sync.dma_start(
                ft.rearrange("p (i c) -> p i c", i=2),
                ft_d.rearrange("i p c -> p i c"))
            pos_i = ft[:, 0:128]    # +S3 * I
            neg_i = ft[:, 128:256]  # -S3 * I

            for g in range(G_PER_CORE):
                for kb in range(NKB):
                    # ev: evacuated W-stage results for one KB block,
                    # layout (s, it, k, e, c, f) so the D stage is two
                    # fully contiguous 4096-elem DVE ops
                    ev = evp.tile([128, KB * 1024], f16, tag="ev")
                    ev7 = ev.rearrange(
                        "p (s it k e c f) -> p it k e s c f",
                        s=2, it=TPB, k=PPT, e=2, c=2)
                    for it in range(TPB):
                        t = inp.tile([128, 2048], f16, tag="xin")
                        nc.sync.dma_start(t[:], x_d[g, kb * TPB + it])
                        # --- H stage on DVE ---
                        a = hop.tile([128, 2048], f16, tag="a")
                        t5 = t.rearrange("p (k s r w) -> p k s r w",
                                         k=PPT, s=2, r=2)
                        a5 = a.rearrange("p (k c s w) -> p k c s w",
                                         k=PPT, c=2, s=2)
                        nc.vector.tensor_add(
                            a5[:, :, 0], t5[:, :, :, 0], t5[:, :, :, 1])
                        nc.vector.tensor_sub(
                            a5[:, :, 1], t5[:, :, :, 0], t5[:, :, :, 1])
                        # --- W stage on TensorE ---
                        # per-kpar PSUM tile (e, s, c, f): each matmul
                        # target is one contiguous 512-elem PSUM bank
                        a7 = a.rearrange(
                            "p (k c s wq f) -> p k s c wq f",
                            k=PPT, c=2, s=2, wq=128)
                        for kpar in range(PPT):
                            pt = psp.tile([128, 1024], f32, tag="ps")
                            rhs_e = a7[:, kpar, :, :, :, 0]
                            rhs_o = a7[:, kpar, :, :, :, 1]
                            lo = pt[:, 0:512]
                            hi = pt[:, 512:1024]
                            nc.tensor.matmul(lo, pos_i, rhs_e,
                                             start=True, stop=False)
                            nc.tensor.matmul(lo, pos_i, rhs_o,
                                             start=False, stop=True)
                            nc.tensor.matmul(hi, pos_i, rhs_e,
                                             start=True, stop=False)
                            nc.tensor.matmul(hi, neg_i, rhs_o,
                                             start=False, stop=True)
                            # --- evac PSUM -> SBUF fp16 (ScalarE) ---
                            nc.scalar.activation(
                                ev7[:, it, kpar],
                                pt.rearrange("p (e s c f) -> p e s c f",
                                             e=2, s=2, c=2),
                                mybir.ActivationFunctionType.Copy)
                    # --- D stage on DVE: two contiguous 4096-elem ops,
                    # each with its own store so the sum half streams
                    # out while the diff half still computes
                    od2 = o_d.rearrange("g p b (d x) -> g p b d x", d=2)
                    sd0 = osp.tile([128, 4096], f16, tag="sd")
                    nc.vector.tensor_add(
                        sd0[:], ev[:, 0:4096], ev[:, 4096:8192])
                    nc.sync.dma_start(od2[g, :, kb, 0], sd0[:])
                    sd1 = osp.tile([128, 4096], f16, tag="sd")
                    nc.vector.tensor_sub(
                        sd1[:], ev[:, 0:4096], ev[:, 4096:8192])
                    nc.sync.dma_start(od2[g, :, kb, 1], sd1[:])
    nc.finalize()
    return nc


def _get_nc():
    if "nc" not in _CACHE:
        _CACHE["nc"] = _build_nc()
    return _CACHE["nc"]


def _unshard_output(results):
    """results: per-core [G, 128, NKB, 8192] fp16 arrays where the
    8192 block is (d, it, k, e, c, f) -> 8 subbands (sb = d*4+c*2+e)
    of [N, C, D/2, H/2, W/2] f32."""
    full = np.stack([np.asarray(r).reshape(
        G_PER_CORE, 128, NKB, 2, TPB, PPT, 2, 2, 128) for r in results])
    # [co, g, h', kb, d, it, k, e, c, f] -> [d, c, e, co, g, kb, it, k, h', f]
    full = full.transpose(4, 8, 7, 0, 1, 3, 5, 6, 2, 9)
    full = full.reshape(8, N, C, KP, 128, 128).astype(np.float32)
    return full


def kernel(x, low_0, low_1, low_2, high_0, high_1, high_2):
    from concourse.bass_utils import run_bass_kernel_spmd

    ft = _build_filter_lhst()
    shards = _shard_input(x)
    in_maps = [{"x": shards[c], "ft": ft} for c in range(NCORES)]
    nc = _get_nc()
    res = run_bass_kernel_spmd(nc, in_maps, list(range(NCORES)))
    full = _unshard_output([res.results[c]["out"] for c in range(NCORES)])
    return tuple(full[s] for s in range(8))


# revision 12
# speedup vs baseline: 2.1652x; 1.0026x over previous
"""3D Haar DWT (depth-1) Trainium2 kernel, fp16 dataflow.

Full inputs: x [4, 4, 64, 256, 256] f32 + six banded Haar matrices.
Every output element is +-2^-1.5 times a +-sum of a 2x2x2 block; the
kernel computes the 3-stage butterfly in fp16 (rel err ~2e-3 vs the
2e-2 gate) to halve DMA traffic, which is the binding resource
(16 DMA engines x 22.5 GB/s per core).

Sharding: data-parallel over N*C = 16 sample-channels, 2 per core on
8 cores. The host pre-transposes each core's input into the exact
SBUF tile layout [g, tile, h', kpar, s, r, w] so every DMA descriptor
is a 4 KiB contiguous run, and un-transposes the packed output
[g, h', kb, sb, q, w'] (16 KiB runs) afterwards.

Per-core pipeline, per 2-pair tile:
  H stage  (DVE):     row pairs r0+-r1, unit-stride fp16 16-bit mode
  W stage  (TensorE): column pairs via accumulating matmuls against
                      +-2^-1.5 * I fp16 stationaries (scale folded in)
  evac     (ScalarE): PSUM -> SBUF fp16 downcast
  D stage  (DVE):     slice pairs s0+-s1 batched over 8 d-pairs
"""
import sys

sys.path.insert(0, "/opt/trn_rl_repo")

import numpy as np

N, C, D, H, W = 4, 4, 64, 256, 256
NCORES = 8
G_PER_CORE = (N * C) // NCORES        # 2
KP = D // 2                           # 32 d-pairs per g
S3 = float(2.0 ** -1.5)

PPT = 8                               # d-pairs per input tile
NT = KP // PPT                        # input tiles per g (16)
KB = 8                                # d-pairs per output staging block
NKB = KP // KB                        # staging blocks per g (4)
TPB = KB // PPT                       # input tiles per staging block (4)

IN_BUFS = 3
A_BUFS = 3
PSUM_BUFS = 4
EV_BUFS = 2
OS_BUFS = 2

_CACHE = {}


def _build_filter_lhst():
    """Stationary operands +-2^-1.5 * I as [2, 128, 128] fp16."""
    eye = np.eye(128, dtype=np.float16)
    return np.stack([np.float16(S3) * eye, -np.float16(S3) * eye])


def _shard_input(x):
    """x [N, C, D, H, W] f32 -> list of per-core [G, NT, 128, PPT*1024]
    fp16 arrays laid out [g, tile, h', kpar, s, r, w]."""
    xs = np.asarray(x).reshape(N * C, D, H, W)
    out = []
    for c in range(NCORES):
        xc = xs[c * G_PER_CORE:(c + 1) * G_PER_CORE]
        # d = tile*2*PPT + kpar*2 + s ; h = h'*2 + r
        v = xc.reshape(G_PER_CORE, NT, PPT, 2, 128, 2, W)
        v = v.transpose(0, 1, 4, 2, 3, 5, 6).astype(np.float16)
        out.append(np.ascontiguousarray(
            v.reshape(G_PER_CORE, NT, 128, PPT * 2 * 2 * W)))
    return out


def _build_nc():
    import concourse.bass as bass
    import concourse.tile as tile
    from concourse import bacc, mybir

    f16 = mybir.dt.float16
    f32 = mybir.dt.float32
    nc = bacc.Bacc(None)
    x_d = nc.declare_dram_parameter("x", [G_PER_CORE, NT, 128, PPT * 1024],
                                    f16, isOutput=False)
    ft_d = nc.declare_dram_parameter("ft", [2, 128, 128], f16,
                                     isOutput=False)
    # packed output: [g, h', kb, (sb q w')] -> 16 KiB run per partition
    o_d = nc.declare_dram_parameter("out", [G_PER_CORE, 128, NKB, 8192],
                                    f16, isOutput=True)

    with tile.TileContext(nc) as tc:
        with (
            tc.tile_pool(name="cst", bufs=1) as cst,
            tc.tile_pool(name="inp", bufs=IN_BUFS) as inp,
            tc.tile_pool(name="hout", bufs=A_BUFS) as hop,
            tc.tile_pool(name="ev", bufs=EV_BUFS) as evp,
            tc.tile_pool(name="os", bufs=OS_BUFS) as osp,
            tc.tile_pool(name="ps", bufs=PSUM_BUFS, space="PSUM") as psp,
        ):
            ft = cst.tile([128, 256], f16, tag="ft")
            nc.sync.dma_start(
                ft.rearrange("p (i c) -> p i c", i=2),
                ft_d.rearrange("i p c -> p i c"))
            pos_i = ft[:, 0:128]    # +S3 * I
            neg_i = ft[:, 128:256]  # -S3 * I

            for g in range(G_PER_CORE):
                for kb in range(NKB):
                    # ev: evacuated W-stage results for one KB block,
                    # layout (s, it, k, e, c, f) so the D stage is two
                    # fully contiguous 4096-elem DVE ops
                    ev = evp.tile([128, KB * 1024], f16, tag="ev")
                    ev7 = ev.rearrange(
                        "p (s it k e c f) -> p it k e s c f",
                        s=2, it=TPB, k=PPT, e=2, c=2)
                    for it in range(TPB):
                        t = inp.tile([128, PPT * 1024], f16, tag="xin")
                        nc.sync.dma_start(t[:], x_d[g, kb * TPB + it])
                        # --- H stage on DVE ---
                        a = hop.tile([128, PPT * 1024], f16, tag="a")
                        t5 = t.rearrange("p (k s r w) -> p k s r w",
                                         k=PPT, s=2, r=2)
                        a5 = a.rearrange("p (k c s w) -> p k c s w",
                                         k=PPT, c=2, s=2)
                        nc.vector.tensor_add(
                            a5[:, :, 0], t5[:, :, :, 0], t5[:, :, :, 1])
                        nc.vector.tensor_sub(
                            a5[:, :, 1], t5[:, :, :, 0], t5[:, :, :, 1])
                        # --- W stage on TensorE ---
                        # per-kpar PSUM tile (e, s, c, f): each matmul
                        # target is one contiguous 512-elem PSUM bank
                        a7 = a.rearrange(
                            "p (k c s wq f) -> p k s c wq f",
                            k=PPT, c=2, s=2, wq=128)
                        for kpar in range(PPT):
                            pt = psp.tile([128, 1024], f32, tag="ps")
                            rhs_e = a7[:, kpar, :, :, :, 0]
                            rhs_o = a7[:, kpar, :, :, :, 1]
                            lo = pt[:, 0:512]
                            hi = pt[:, 512:1024]
                            nc.tensor.matmul(lo, pos_i, rhs_e,
                                             start=True, stop=False)
                            nc.tensor.matmul(lo, pos_i, rhs_o,
                                             start=False, stop=True)
                            nc.tensor.matmul(hi, pos_i, rhs_e,
                                             start=True, stop=False)
                            nc.tensor.matmul(hi, neg_i, rhs_o,
                                             start=False, stop=True)
                            # --- evac PSUM -> SBUF fp16 (ScalarE) ---
                            nc.scalar.activation(
                                ev7[:, it, kpar],
                                pt.rearrange("p (e s c f) -> p e s c f",
                                             e=2, s=2, c=2),
                                mybir.ActivationFunctionType.Copy)
                    # --- D stage on DVE: two contiguous 4096-elem ops
                    sd = osp.tile([128, 8192], f16, tag="sd")
                    nc.vector.tensor_add(
                        sd[:, 0:4096], ev[:, 0:4096], ev[:, 4096:8192])
                    nc.vector.tensor_sub(
                        sd[:, 4096:8192], ev[:, 0:4096], ev[:, 4096:8192])
                    nc.sync.dma_start(o_d[g, :, kb], sd[:])
    nc.finalize()
    return nc


def _get_nc():
    if "nc" not in _CACHE:
        _CACHE["nc"] = _build_nc()
    return _CACHE["nc"]


def _unshard_output(results):
    """results: per-core [G, 128, NKB, 8192] fp16 arrays where the
    8192 block is (d, it, k, e, c, f) -> 8 subbands (sb = d*4+c*2+e)
    of [N, C, D/2, H/2, W/2] f32."""
    full = np.stack([np.asarray(r).reshape(
        G_PER_CORE, 128, NKB, 2, TPB, PPT, 2, 2, 128) for r in results])
    # [co, g, h', kb, d, it, k, e, c, f] -> [d, c, e, co, g, kb, it, k, h', f]
    full = full.transpose(4, 8, 7, 0, 1, 3, 5, 6, 2, 9)
    full = full.reshape(8, N, C, KP, 128, 128).astype(np.float32)
    return full


def kernel(x, low_0, low_1, low_2, high_0, high_1, high_2):
    from concourse.bass_utils import run_bass_kernel_spmd

    ft = _build_filter_lhst()
    shards = _shard_input(x)
    in_maps = [{"x": shards[c], "ft": ft} for c in range(NCORES)]
    nc = _get_nc()
    res = run_bass_kernel_spmd(nc, in_maps, list(range(NCORES)))
    full = _unshard_output([res.results[c]["out"] for c in range(NCORES)])
    return tuple(full[s] for s in range(8))


# revision 15
# speedup vs baseline: 2.1677x; 1.0012x over previous
"""3D Haar DWT (depth-1) Trainium2 kernel, fp16 dataflow.

Full inputs: x [4, 4, 64, 256, 256] f32 + six banded Haar matrices.
Every output element is +-2^-1.5 times a +-sum of a 2x2x2 block; the
kernel computes the 3-stage butterfly in fp16 (rel err ~2e-3 vs the
2e-2 gate) to halve DMA traffic, which is the binding resource
(16 DMA engines x 22.5 GB/s per core).

Sharding: data-parallel over N*C = 16 sample-channels, 2 per core on
8 cores. The host pre-transposes each core's input into the exact
SBUF tile layout [g, tile, h', kpar, s, r, w] so every DMA descriptor
is a 4 KiB contiguous run, and un-transposes the packed output
[g, h', kb, sb, q, w'] (16 KiB runs) afterwards.

Per-core pipeline, per 2-pair tile:
  H stage  (DVE):     row pairs r0+-r1, unit-stride fp16 16-bit mode
  W stage  (TensorE): column pairs via accumulating matmuls against
                      +-2^-1.5 * I fp16 stationaries (scale folded in)
  evac     (ScalarE): PSUM -> SBUF fp16 downcast
  D stage  (DVE):     slice pairs s0+-s1 batched over 8 d-pairs
"""
import sys

sys.path.insert(0, "/opt/trn_rl_repo")

import numpy as np

N, C, D, H, W = 4, 4, 64, 256, 256
NCORES = 8
G_PER_CORE = (N * C) // NCORES        # 2
KP = D // 2                           # 32 d-pairs per g
S3 = float(2.0 ** -1.5)

PPT = 8                               # d-pairs per input tile
NT = KP // PPT                        # input tiles per g (16)
KB = 8                                # d-pairs per output staging block
NKB = KP // KB                        # staging blocks per g (4)
TPB = KB // PPT                       # input tiles per staging block (4)

IN_BUFS = 4
A_BUFS = 3
PSUM_BUFS = 4
EV_BUFS = 2
OS_BUFS = 2

_CACHE = {}


def _build_filter_lhst():
    """Stationary operands +-2^-1.5 * I as [2, 128, 128] fp16."""
    eye = np.eye(128, dtype=np.float16)
    return np.stack([np.float16(S3) * eye, -np.float16(S3) * eye])


def _shard_input(x):
    """x [N, C, D, H, W] f32 -> list of per-core [G, NT, 128, PPT*1024]
    fp16 arrays laid out [g, tile, h', kpar, s, r, w]."""
    xs = np.asarray(x).reshape(N * C, D, H, W)
    out = []
    for c in range(NCORES):
        xc = xs[c * G_PER_CORE:(c + 1) * G_PER_CORE]
        # d = tile*2*PPT + kpar*2 + s ; h = h'*2 + r
        v = xc.reshape(G_PER_CORE, NT, PPT, 2, 128, 2, W)
        v = v.transpose(0, 1, 4, 2, 3, 5, 6).astype(np.float16)
        out.append(np.ascontiguousarray(
            v.reshape(G_PER_CORE, NT, 128, PPT * 2 * 2 * W)))
    return out


def _build_nc():
    import concourse.bass as bass
    import concourse.tile as tile
    from concourse import bacc, mybir

    f16 = mybir.dt.float16
    f32 = mybir.dt.float32
    nc = bacc.Bacc(None)
    x_d = nc.declare_dram_parameter("x", [G_PER_CORE, NT, 128, PPT * 1024],
                                    f16, isOutput=False)
    ft_d = nc.declare_dram_parameter("ft", [2, 128, 128], f16,
                                     isOutput=False)
    # packed output: [g, h', kb, (sb q w')] -> 16 KiB run per partition
    o_d = nc.declare_dram_parameter("out", [G_PER_CORE, 128, NKB, 8192],
                                    f16, isOutput=True)

    with tile.TileContext(nc) as tc:
        with (
            tc.tile_pool(name="cst", bufs=1) as cst,
            tc.tile_pool(name="inp", bufs=IN_BUFS) as inp,
            tc.tile_pool(name="hout", bufs=A_BUFS) as hop,
            tc.tile_pool(name="ev", bufs=EV_BUFS) as evp,
            tc.tile_pool(name="os", bufs=OS_BUFS) as osp,
            tc.tile_pool(name="ps", bufs=PSUM_BUFS, space="PSUM") as psp,
        ):
            ft = cst.tile([128, 256], f16, tag="ft")
            nc.sync.dma_start(
                ft.rearrange("p (i c) -> p i c", i=2),
                ft_d.rearrange("i p c -> p i c"))
            pos_i = ft[:, 0:128]    # +S3 * I
            neg_i = ft[:, 128:256]  # -S3 * I

            for g in range(G_PER_CORE):
                for kb in range(NKB):
                    # ev: evacuated W-stage results for one KB block,
                    # layout (s, it, k, e, c, f) so the D stage is two
                    # fully contiguous 4096-elem DVE ops
                    ev = evp.tile([128, KB * 1024], f16, tag="ev")
                    ev7 = ev.rearrange(
                        "p (s it k e c f) -> p it k e s c f",
                        s=2, it=TPB, k=PPT, e=2, c=2)
                    for it in range(TPB):
                        t = inp.tile([128, PPT * 1024], f16, tag="xin")
                        nc.sync.dma_start(t[:], x_d[g, kb * TPB + it])
                        # --- H stage on DVE ---
                        a = hop.tile([128, PPT * 1024], f16, tag="a")
                        t5 = t.rearrange("p (k s r w) -> p k s r w",
                                         k=PPT, s=2, r=2)
                        a5 = a.rearrange("p (k c s w) -> p k c s w",
                                         k=PPT, c=2, s=2)
                        nc.vector.tensor_add(
                            a5[:, :, 0], t5[:, :, :, 0], t5[:, :, :, 1])
                        nc.vector.tensor_sub(
                            a5[:, :, 1], t5[:, :, :, 0], t5[:, :, :, 1])
                        # --- W stage on TensorE ---
                        # per-kpar PSUM tile (e, s, c, f): each matmul
                        # target is one contiguous 512-elem PSUM bank
                        a7 = a.rearrange(
                            "p (k c s wq f) -> p k s c wq f",
                            k=PPT, c=2, s=2, wq=128)
                        for kpar in range(PPT):
                            pt = psp.tile([128, 1024], f32, tag="ps")
                            rhs_e = a7[:, kpar, :, :, :, 0]
                            rhs_o = a7[:, kpar, :, :, :, 1]
                            lo = pt[:, 0:512]
                            hi = pt[:, 512:1024]
                            nc.tensor.matmul(lo, pos_i, rhs_e,
                                             start=True, stop=False)
                            nc.tensor.matmul(lo, pos_i, rhs_o,
                                             start=False, stop=True)
                            nc.tensor.matmul(hi, pos_i, rhs_e,
                                             start=True, stop=False)
                            nc.tensor.matmul(hi, neg_i, rhs_o,
                                             start=False, stop=True)
                            # --- evac PSUM -> SBUF fp16 (ScalarE) ---
                            nc.scalar.activation(
                                ev7[:, it, kpar],
                                pt.rearrange("p (e s c f) -> p e s c f",
                                             e=2, s=2, c=2),
                                mybir.ActivationFunctionType.Copy)
                    # --- D stage on DVE: two contiguous 4096-elem ops
                    sd = osp.tile([128, 8192], f16, tag="sd")
                    nc.vector.tensor_add(
                        sd[:, 0:4096], ev[:, 0:4096], ev[:, 4096:8192])
                    nc.vector.tensor_sub(
                        sd[:, 4096:8192], ev[:, 0:4096], ev[:, 4096:8192])
                    # store from the (otherwise idle) Pool queue so it
                    # cannot head-of-line-block the SP load stream
                    nc.gpsimd.dma_start(o_d[g, :, kb], sd[:])
    nc.finalize()
    return nc


def _get_nc():
    if "nc" not in _CACHE:
        _CACHE["nc"] = _build_nc()
    return _CACHE["nc"]


def _unshard_output(results):
    """results: per-core [G, 128, NKB, 8192] fp16 arrays where the
    8192 block is (d, it, k, e, c, f) -> 8 subbands (sb = d*4+c*2+e)
    of [N, C, D/2, H/2, W/2] f32."""
    full = np.stack([np.asarray(r).reshape(
        G_PER_CORE, 128, NKB, 2, TPB, PPT, 2, 2, 128) for r in results])
    # [co, g, h', kb, d, it, k, e, c, f] -> [d, c, e, co, g, kb, it, k, h', f]
    full = full.transpose(4, 8, 7, 0, 1, 3, 5, 6, 2, 9)
    full = full.reshape(8, N, C, KP, 128, 128).astype(np.float32)
    return full


def kernel(x, low_0, low_1, low_2, high_0, high_1, high_2):
    from concourse.bass_utils import run_bass_kernel_spmd

    ft = _build_filter_lhst()
    shards = _shard_input(x)
    in_maps = [{"x": shards[c], "ft": ft} for c in range(NCORES)]
    nc = _get_nc()
    res = run_bass_kernel_spmd(nc, in_maps, list(range(NCORES)))
    full = _unshard_output([res.results[c]["out"] for c in range(NCORES)])
    return tuple(full[s] for s in range(8))


# revision 17
# speedup vs baseline: 2.2638x; 1.0443x over previous
"""3D Haar DWT (depth-1) Trainium2 kernel, fp16 dataflow.

Full inputs: x [4, 4, 64, 256, 256] f32 + six banded Haar matrices.
Every output element is +-2^-1.5 times a +-sum of a 2x2x2 block; the
kernel computes the 3-stage butterfly in fp16 (rel err ~2e-3 vs the
2e-2 gate) to halve DMA traffic, which is the binding resource
(16 DMA engines x 22.5 GB/s per core).

Sharding: data-parallel over N*C = 16 sample-channels, 2 per core on
8 cores. The host pre-transposes each core's input into the exact
SBUF tile layout [g, tile, h', kpar, s, r, w] so every DMA descriptor
is a 4 KiB contiguous run, and un-transposes the packed output
[g, h', kb, sb, q, w'] (16 KiB runs) afterwards.

Per-core pipeline, per 2-pair tile:
  H stage  (DVE):     row pairs r0+-r1, unit-stride fp16 16-bit mode
  W stage  (TensorE): column pairs via accumulating matmuls against
                      +-2^-1.5 * I fp16 stationaries (scale folded in)
  evac     (ScalarE): PSUM -> SBUF fp16 downcast
  D stage  (DVE):     slice pairs s0+-s1 batched over 8 d-pairs
"""
import sys

sys.path.insert(0, "/opt/trn_rl_repo")

import numpy as np

N, C, D, H, W = 4, 4, 64, 256, 256
NCORES = 8
G_PER_CORE = (N * C) // NCORES        # 2
KP = D // 2                           # 32 d-pairs per g
S3 = float(2.0 ** -1.5)

PPT = 8                               # d-pairs per input tile
NT = KP // PPT                        # input tiles per g (16)
KB = 8                                # d-pairs per output staging block
NKB = KP // KB                        # staging blocks per g (4)
TPB = KB // PPT                       # input tiles per staging block (4)

IN_BUFS = 4
A_BUFS = 3
PSUM_BUFS = 4
EV_BUFS = 2
OS_BUFS = 2

_CACHE = {}


def _build_filter_lhst():
    """Stationary operands +-2^-1.5 * I as [2, 128, 128] fp16."""
    eye = np.eye(128, dtype=np.float16)
    return np.stack([np.float16(S3) * eye, -np.float16(S3) * eye])


def _shard_input(x):
    """x [N, C, D, H, W] f32 -> list of per-core [G, NT, 128, PPT*1024]
    fp16 arrays laid out [g, tile, h', kpar, s, r, w]."""
    xs = np.asarray(x).reshape(N * C, D, H, W)
    out = []
    for c in range(NCORES):
        xc = xs[c * G_PER_CORE:(c + 1) * G_PER_CORE]
        # d = tile*2*PPT + kpar*2 + s ; h = h'*2 + r
        v = xc.reshape(G_PER_CORE, NT, PPT, 2, 128, 2, W)
        v = v.transpose(0, 1, 4, 2, 3, 5, 6).astype(np.float16)
        out.append(np.ascontiguousarray(
            v.reshape(G_PER_CORE, NT, 128, PPT * 2 * 2 * W)))
    return out


def _build_nc():
    import concourse.bass as bass
    import concourse.tile as tile
    from concourse import bacc, mybir

    f16 = mybir.dt.float16
    f32 = mybir.dt.float32
    nc = bacc.Bacc(None)
    x_d = nc.declare_dram_parameter("x", [G_PER_CORE, NT, 128, PPT * 1024],
                                    f16, isOutput=False)
    ft_d = nc.declare_dram_parameter("ft", [2, 128, 128], f16,
                                     isOutput=False)
    # packed output: [g, h', kb, (sb q w')] -> 16 KiB run per partition
    o_d = nc.declare_dram_parameter("out", [G_PER_CORE, 128, NKB, 8192],
                                    f16, isOutput=True)

    with tile.TileContext(nc) as tc:
        with (
            tc.tile_pool(name="cst", bufs=1) as cst,
            tc.tile_pool(name="inp", bufs=IN_BUFS) as inp,
            tc.tile_pool(name="hout", bufs=A_BUFS) as hop,
            tc.tile_pool(name="ev", bufs=EV_BUFS) as evp,
            tc.tile_pool(name="os", bufs=OS_BUFS) as osp,
            tc.tile_pool(name="ps", bufs=PSUM_BUFS, space="PSUM") as psp,
        ):
            ft = cst.tile([128, 256], f16, tag="ft")
            nc.sync.dma_start(
                ft.rearrange("p (i c) -> p i c", i=2),
                ft_d.rearrange("i p c -> p i c"))
            pos_i = ft[:, 0:128]    # +S3 * I
            neg_i = ft[:, 128:256]  # -S3 * I

            for g in range(G_PER_CORE):
                for kb in range(NKB):
                    # ev: evacuated W-stage results for one KB block,
                    # layout (s, it, k, e, c, f) so the D stage is two
                    # fully contiguous 4096-elem DVE ops
                    ev = evp.tile([128, KB * 1024], f16, tag="ev")
                    ev7 = ev.rearrange(
                        "p (s it k e c f) -> p it k e s c f",
                        s=2, it=TPB, k=PPT, e=2, c=2)
                    for it in range(TPB):
                        t = inp.tile([128, PPT * 1024], f16, tag="xin")
                        nc.sync.dma_start(t[:], x_d[g, kb * TPB + it])
                        # --- H stage on DVE ---
                        a = hop.tile([128, PPT * 1024], f16, tag="a")
                        t5 = t.rearrange("p (k s r w) -> p k s r w",
                                         k=PPT, s=2, r=2)
                        a5 = a.rearrange("p (k c s w) -> p k c s w",
                                         k=PPT, c=2, s=2)
                        # split per k-half so the first matmuls can
                        # start before the whole tile's H is done
                        hh = PPT // 2
                        for k0 in (0, hh):
                            ksl = slice(k0, k0 + hh)
                            nc.vector.tensor_add(
                                a5[:, ksl, 0], t5[:, ksl, :, 0],
                                t5[:, ksl, :, 1])
                            nc.vector.tensor_sub(
                                a5[:, ksl, 1], t5[:, ksl, :, 0],
                                t5[:, ksl, :, 1])
                        # --- W stage on TensorE ---
                        # per-kpar PSUM tile (e, s, c, f): each matmul
                        # target is one contiguous 512-elem PSUM bank
                        a7 = a.rearrange(
                            "p (k c s wq f) -> p k s c wq f",
                            k=PPT, c=2, s=2, wq=128)
                        for kpar in range(PPT):
                            pt = psp.tile([128, 1024], f32, tag="ps")
                            rhs_e = a7[:, kpar, :, :, :, 0]
                            rhs_o = a7[:, kpar, :, :, :, 1]
                            lo = pt[:, 0:512]
                            hi = pt[:, 512:1024]
                            nc.tensor.matmul(lo, pos_i, rhs_e,
                                             start=True, stop=False)
                            nc.tensor.matmul(lo, pos_i, rhs_o,
                                             start=False, stop=True)
                            nc.tensor.matmul(hi, pos_i, rhs_e,
                                             start=True, stop=False)
                            nc.tensor.matmul(hi, neg_i, rhs_o,
                                             start=False, stop=True)
                            # --- evac PSUM -> SBUF fp16 (ScalarE) ---
                            nc.scalar.activation(
                                ev7[:, it, kpar],
                                pt.rearrange("p (e s c f) -> p e s c f",
                                             e=2, s=2, c=2),
                                mybir.ActivationFunctionType.Copy)
                    # --- D stage on DVE: contiguous ops, split per
                    # k-half so they fire as soon as 4 evacs finish
                    sd = osp.tile([128, 8192], f16, tag="sd")
                    for o0 in (0, 2048):
                        nc.vector.tensor_add(
                            sd[:, o0:o0 + 2048], ev[:, o0:o0 + 2048],
                            ev[:, o0 + 4096:o0 + 6144])
                        nc.vector.tensor_sub(
                            sd[:, o0 + 4096:o0 + 6144], ev[:, o0:o0 + 2048],
                            ev[:, o0 + 4096:o0 + 6144])
                    # store from the (otherwise idle) Pool queue so it
                    # cannot head-of-line-block the SP load stream
                    nc.gpsimd.dma_start(o_d[g, :, kb], sd[:])
    nc.finalize()
    return nc


def _get_nc():
    if "nc" not in _CACHE:
        _CACHE["nc"] = _build_nc()
    return _CACHE["nc"]


def _unshard_output(results):
    """results: per-core [G, 128, NKB, 8192] fp16 arrays where the
    8192 block is (d, it, k, e, c, f) -> 8 subbands (sb = d*4+c*2+e)
    of [N, C, D/2, H/2, W/2] f32."""
    full = np.stack([np.asarray(r).reshape(
        G_PER_CORE, 128, NKB, 2, TPB, PPT, 2, 2, 128) for r in results])
    # [co, g, h', kb, d, it, k, e, c, f] -> [d, c, e, co, g, kb, it, k, h', f]
    full = full.transpose(4, 8, 7, 0, 1, 3, 5, 6, 2, 9)
    full = full.reshape(8, N, C, KP, 128, 128).astype(np.float32)
    return full


def kernel(x, low_0, low_1, low_2, high_0, high_1, high_2):
    from concourse.bass_utils import run_bass_kernel_spmd

    ft = _build_filter_lhst()
    shards = _shard_input(x)
    in_maps = [{"x": shards[c], "ft": ft} for c in range(NCORES)]
    nc = _get_nc()
    res = run_bass_kernel_spmd(nc, in_maps, list(range(NCORES)))
    full = _unshard_output([res.results[c]["out"] for c in range(NCORES)])
    return tuple(full[s] for s in range(8))


# revision 18
# speedup vs baseline: 2.2825x; 1.0083x over previous
"""3D Haar DWT (depth-1) Trainium2 kernel, fp16 dataflow.

Full inputs: x [4, 4, 64, 256, 256] f32 + six banded Haar matrices.
Every output element is +-2^-1.5 times a +-sum of a 2x2x2 block; the
kernel computes the 3-stage butterfly in fp16 (rel err ~2e-3 vs the
2e-2 gate) to halve DMA traffic, which is the binding resource
(16 DMA engines x 22.5 GB/s per core).

Sharding: data-parallel over N*C = 16 sample-channels, 2 per core on
8 cores. The host pre-transposes each core's input into the exact
SBUF tile layout [g, tile, h', kpar, s, r, w] so every DMA descriptor
is a 4 KiB contiguous run, and un-transposes the packed output
[g, h', kb, sb, q, w'] (16 KiB runs) afterwards.

Per-core pipeline, per 2-pair tile:
  H stage  (DVE):     row pairs r0+-r1, unit-stride fp16 16-bit mode
  W stage  (TensorE): column pairs via accumulating matmuls against
                      +-2^-1.5 * I fp16 stationaries (scale folded in)
  evac     (ScalarE): PSUM -> SBUF fp16 downcast
  D stage  (DVE):     slice pairs s0+-s1 batched over 8 d-pairs
"""
import sys

sys.path.insert(0, "/opt/trn_rl_repo")

import numpy as np

N, C, D, H, W = 4, 4, 64, 256, 256
NCORES = 8
G_PER_CORE = (N * C) // NCORES        # 2
KP = D // 2                           # 32 d-pairs per g
S3 = float(2.0 ** -1.5)

PPT = 8                               # d-pairs per input tile
NT = KP // PPT                        # input tiles per g (16)
KB = 8                                # d-pairs per output staging block
NKB = KP // KB                        # staging blocks per g (4)
TPB = KB // PPT                       # input tiles per staging block (4)

IN_BUFS = 2
A_BUFS = 3
PSUM_BUFS = 4
EV_BUFS = 2
OS_BUFS = 4

_CACHE = {}


def _build_filter_lhst():
    """Stationary operands +-2^-1.5 * I as [2, 128, 128] fp16."""
    eye = np.eye(128, dtype=np.float16)
    return np.stack([np.float16(S3) * eye, -np.float16(S3) * eye])


def _shard_input(x):
    """x [N, C, D, H, W] f32 -> list of per-core [G, NT, 128, PPT*1024]
    fp16 arrays laid out [g, tile, h', kpar, s, r, w]."""
    xs = np.asarray(x).reshape(N * C, D, H, W)
    out = []
    for c in range(NCORES):
        xc = xs[c * G_PER_CORE:(c + 1) * G_PER_CORE]
        # d = tile*2*PPT + kpar*2 + s ; h = h'*2 + r
        v = xc.reshape(G_PER_CORE, NT, PPT, 2, 128, 2, W)
        v = v.transpose(0, 1, 4, 2, 3, 5, 6).astype(np.float16)
        out.append(np.ascontiguousarray(
            v.reshape(G_PER_CORE, NT, 128, PPT * 2 * 2 * W)))
    return out


def _build_nc():
    import concourse.bass as bass
    import concourse.tile as tile
    from concourse import bacc, mybir

    f16 = mybir.dt.float16
    f32 = mybir.dt.float32
    nc = bacc.Bacc(None)
    x_d = nc.declare_dram_parameter("x", [G_PER_CORE, NT, 128, PPT * 1024],
                                    f16, isOutput=False)
    ft_d = nc.declare_dram_parameter("ft", [2, 128, 128], f16,
                                     isOutput=False)
    # packed output: [g, h', kb, (sb q w')] -> 16 KiB run per partition
    o_d = nc.declare_dram_parameter("out", [G_PER_CORE, 128, NKB, 8192],
                                    f16, isOutput=True)

    with tile.TileContext(nc) as tc:
        with (
            tc.tile_pool(name="cst", bufs=1) as cst,
            tc.tile_pool(name="inp", bufs=IN_BUFS) as inp,
            tc.tile_pool(name="hout", bufs=A_BUFS) as hop,
            tc.tile_pool(name="ev", bufs=EV_BUFS) as evp,
            tc.tile_pool(name="os", bufs=OS_BUFS) as osp,
            tc.tile_pool(name="ps", bufs=PSUM_BUFS, space="PSUM") as psp,
        ):
            ft = cst.tile([128, 256], f16, tag="ft")
            nc.sync.dma_start(
                ft.rearrange("p (i c) -> p i c", i=2),
                ft_d.rearrange("i p c -> p i c"))
            pos_i = ft[:, 0:128]    # +S3 * I
            neg_i = ft[:, 128:256]  # -S3 * I

            for g in range(G_PER_CORE):
                for kb in range(NKB):
                    # ev: evacuated W-stage results for one KB block,
                    # layout (s, it, k, e, c, f) so the D stage is two
                    # fully contiguous 4096-elem DVE ops
                    ev = evp.tile([128, KB * 1024], f16, tag="ev")
                    ev7 = ev.rearrange(
                        "p (s it k e c f) -> p it k e s c f",
                        s=2, it=TPB, k=PPT, e=2, c=2)
                    for it in range(TPB):
                        t = inp.tile([128, PPT * 1024], f16, tag="xin")
                        nc.sync.dma_start(t[:], x_d[g, kb * TPB + it])
                        # --- H stage on DVE ---
                        a = hop.tile([128, PPT * 1024], f16, tag="a")
                        t5 = t.rearrange("p (k s r w) -> p k s r w",
                                         k=PPT, s=2, r=2)
                        a5 = a.rearrange("p (k c s w) -> p k c s w",
                                         k=PPT, c=2, s=2)
                        # split per k-half so the first matmuls can
                        # start before the whole tile's H is done
                        hh = PPT // 2
                        for k0 in (0, hh):
                            ksl = slice(k0, k0 + hh)
                            nc.vector.tensor_add(
                                a5[:, ksl, 0], t5[:, ksl, :, 0],
                                t5[:, ksl, :, 1])
                            nc.vector.tensor_sub(
                                a5[:, ksl, 1], t5[:, ksl, :, 0],
                                t5[:, ksl, :, 1])
                        # --- W stage on TensorE ---
                        # per-kpar PSUM tile (e, s, c, f): each matmul
                        # target is one contiguous 512-elem PSUM bank
                        a7 = a.rearrange(
                            "p (k c s wq f) -> p k s c wq f",
                            k=PPT, c=2, s=2, wq=128)
                        for kpar in range(PPT):
                            pt = psp.tile([128, 1024], f32, tag="ps")
                            rhs_e = a7[:, kpar, :, :, :, 0]
                            rhs_o = a7[:, kpar, :, :, :, 1]
                            lo = pt[:, 0:512]
                            hi = pt[:, 512:1024]
                            nc.tensor.matmul(lo, pos_i, rhs_e,
                                             start=True, stop=False)
                            nc.tensor.matmul(lo, pos_i, rhs_o,
                                             start=False, stop=True)
                            nc.tensor.matmul(hi, pos_i, rhs_e,
                                             start=True, stop=False)
                            nc.tensor.matmul(hi, neg_i, rhs_o,
                                             start=False, stop=True)
                            # --- evac PSUM -> SBUF fp16 (ScalarE) ---
                            nc.scalar.activation(
                                ev7[:, it, kpar],
                                pt.rearrange("p (e s c f) -> p e s c f",
                                             e=2, s=2, c=2),
                                mybir.ActivationFunctionType.Copy)
                    # --- D stage on DVE: contiguous ops, split per
                    # k-half so they fire as soon as 4 evacs finish
                    sd = osp.tile([128, 8192], f16, tag="sd")
                    for o0 in (0, 2048):
                        nc.vector.tensor_add(
                            sd[:, o0:o0 + 2048], ev[:, o0:o0 + 2048],
                            ev[:, o0 + 4096:o0 + 6144])
                        nc.vector.tensor_sub(
                            sd[:, o0 + 4096:o0 + 6144], ev[:, o0:o0 + 2048],
                            ev[:, o0 + 4096:o0 + 6144])
                    # store from the (otherwise idle) Pool queue so it
                    # cannot head-of-line-block the SP load stream
                    nc.gpsimd.dma_start(o_d[g, :, kb], sd[:])
    nc.finalize()
    return nc


def _get_nc():
    if "nc" not in _CACHE:
        _CACHE["nc"] = _build_nc()
    return _CACHE["nc"]


def _unshard_output(results):
    """results: per-core [G, 128, NKB, 8192] fp16 arrays where the
    8192 block is (d, it, k, e, c, f) -> 8 subbands (sb = d*4+c*2+e)
    of [N, C, D/2, H/2, W/2] f32."""
    full = np.stack([np.asarray(r).reshape(
        G_PER_CORE, 128, NKB, 2, TPB, PPT, 2, 2, 128) for r in results])
    # [co, g, h', kb, d, it, k, e, c, f] -> [d, c, e, co, g, kb, it, k, h', f]
    full = full.transpose(4, 8, 7, 0, 1, 3, 5, 6, 2, 9)
    full = full.reshape(8, N, C, KP, 128, 128).astype(np.float32)
    return full


def kernel(x, low_0, low_1, low_2, high_0, high_1, high_2):
    from concourse.bass_utils import run_bass_kernel_spmd

    ft = _build_filter_lhst()
    shards = _shard_input(x)
    in_maps = [{"x": shards[c], "ft": ft} for c in range(NCORES)]
    nc = _get_nc()
    res = run_bass_kernel_spmd(nc, in_maps, list(range(NCORES)))
    full = _unshard_output([res.results[c]["out"] for c in range(NCORES)])
    return tuple(full[s] for s in range(8))
